# revision 35
# baseline (speedup 1.0000x reference)
"""Trainium2 Bass kernel for AdaptivePositionAwareAttention.

Banded softmax + closed-form far field (see reference): pos_w = exp(-|i-j|/2)
kills exp(f)-1 beyond ~90 positions, so per 128-row query block t only key
blocks t-1..t+1 (WIN=384) matter:

    y_i = zr_i * (VsumWo + sum_band (e^{f_ij}-1) VW_j) + (bv@Wo + bo)

where VW = xp @ (Wo Wv)^T (Wo folded into Wv on host), VsumWo = column-sum of
VW over the full sequence (cross-half part via one AllGather), and
zr_i = 1/(S - WIN + sum_band e^f).  The task/content/fusion branches reduce to
a per-row scalar g with f = base * g.

Layout/dtype scheme: x+pos loaded fp16, added on DVE, transposed feature-major
by the DMA XBAR (one 3D-AP DmaTranspose per block), cast to fp8.  The heavy
projections (K, Q, h1, VW) and scores run as fp8 DoubleRow matmuls (2 k-planes
per pass via 3D APs); E'@VW runs bf16.  zr is folded into E' so the output
needs only one vector add per block.

Sharding: 8 cores = (batch 0..3) x (sequence half); each core owns 1024 query
rows plus one 128-row zero-padded halo block per side.  Sequence-edge effects
ride on per-core masked pos_w *data*, so all cores run one SPMD graph.
"""

import math

import numpy as np
import ml_dtypes

import concourse.bass as bass
from concourse import bacc, mybir, tile
from concourse.bass_utils import run_bass_kernel_spmd

B, S, H = 4, 2048, 768
HC = H // 128             # 6 feature chunks
NB = 10                   # halo blocks per core (1280 rows)
WIN = 384                 # key window = 3 blocks
F32 = mybir.dt.float32
F16 = mybir.dt.float16
BF16 = mybir.dt.bfloat16
F8 = mybir.dt.float8e4
AF = mybir.ActivationFunctionType
ALU = mybir.AluOpType
AX = mybir.AxisListType
DR = mybir.MatmulPerfMode.DoubleRow

WVO_SC = 16.0             # host scale on (Wo Wv)^T to dodge fp8 subnormals
WC3_SC = 64.0
FU1_SC = 4096.0           # fu1s eviction scale (values ~3e-5 raw)
WF2_SC = 64.0

_cache = {}


def r3(ap, c):
    return ap.rearrange("p (c s) -> p c s", c=c)


def build_kernel(dbg=False):
    nc = bacc.Bacc(None, target_bir_lowering=False)

    def din(name, shape, dt=F32):
        return nc.dram_tensor(name, shape, dt, kind="ExternalInput")

    xp2 = din("xp2", [NB * 128, 2 * H], BF16)   # rows: [x | pos], zero-padded
    wq8 = din("wq8", [H, H], F8)                # Wq^T  [hi, ho]
    wk8 = din("wk8", [H, H], F8)
    wvo8 = din("wvo8", [H, H], F8)              # (Wo Wv)^T * WVO_SC
    wvob = din("wvob", [H, H], BF16)            # (Wo Wv)^T exact-ish
    wc18 = din("wc18", [H, 384], F8)            # Wc1^T
    wc28 = din("wc28", [384, 256], F8)          # Wc2^T zero-padded 192->256
    wt2b = din("wt2b", [H, H], BF16)            # Wt2^T
    wtep = din("wtep", [64, H + 1], BF16)       # [Wt1^T | te]
    wf1p = din("wf1p", [3, H + 1], BF16)        # [Wf1^T / S | ones col]
    f8p = din("f8p", [128, 8], F8)              # cols 0-5 Wf2*64, 6-7 Wc3*64
    smf = din("smf", [128, 26], F32)            # packed per-partition consts
    # smf cols: 0-5 bq, 6-11 bk, 12-14 bc1, 15-16 bc2, 17-22 bf1*FU1_SC,
    #           23 bc3 (bcast), 24 bf2 (bcast), 25 unused
    # bfp cols: 0:128 ident, 128 ones col, 129+ 3*WIN pw f/m/l, then wf2 [128,6],
    # then row-0-only: ones_row[128], bvoo[768], bt1|bt2[1536]
    NBF = 129 + 3 * WIN + 6 + 128 + H + 2 * H
    bfp = din("bfp", [128, NBF], BF16)

    y = nc.dram_tensor("y", [1024, H], BF16, kind="ExternalOutput")
    if dbg:
        d_ts = nc.dram_tensor("d_ts", [1, 1], F32, kind="ExternalOutput")
        d_ci = nc.dram_tensor("d_ci", [1, 1024], F32, kind="ExternalOutput")
        d_g8 = nc.dram_tensor("d_g8", [128, 8], F32, kind="ExternalOutput")
        d_zr8 = nc.dram_tensor("d_zr8", [128, 8], F32, kind="ExternalOutput")
        d_vswo = nc.dram_tensor("d_vswo", [1, H], F32, kind="ExternalOutput")
        d_rs8 = nc.dram_tensor("d_rs8", [128, 8], F32, kind="ExternalOutput")
        d_fw = nc.dram_tensor("d_fw", [1, 1024], F32, kind="ExternalOutput")
        d_kt = nc.dram_tensor("d_kt", [128, 128], F32, kind="ExternalOutput")
        d_vw = nc.dram_tensor("d_vw", [128, H], F32, kind="ExternalOutput")
        d_q = nc.dram_tensor("d_q", [128, 128], F32, kind="ExternalOutput")
        d_x8 = nc.dram_tensor("d_x8", [128, 256], F32, kind="ExternalOutput")
        d_wk = nc.dram_tensor("d_wk", [128, 256], F32, kind="ExternalOutput")
        d_bp = nc.dram_tensor("d_bp", [128, 384], F32, kind="ExternalOutput")

    with tile.TileContext(nc) as tc:
        with (
            tc.tile_pool(name="pers", bufs=1) as pers,
            tc.tile_pool(name="wp", bufs=4) as wp,
            tc.tile_pool(name="wbp", bufs=1) as wbp,
            tc.tile_pool(name="pA", bufs=3, space="PSUM") as pA,
            tc.tile_pool(name="pT", bufs=2, space="PSUM") as pT,
            tc.tile_pool(name="pB", bufs=2, space="PSUM") as pB,
            tc.tile_pool(name="pvs", bufs=1, space="PSUM") as pvs,
        ):
            # ---------- small packed loads ----------
            smf_t = pers.tile([128, 26], F32, tag="smf")
            nc.sync.dma_start(smf_t[:], smf[:])
            bfp_t = pers.tile([128, NBF], BF16, tag="bfp")
            nc.sync.dma_start(bfp_t[:, 0:NBF // 2], bfp[:, 0:NBF // 2])
            nc.scalar.dma_start(bfp_t[:, NBF // 2:], bfp[:, NBF // 2:])
            identb = bfp_t[:, 0:128]
            ones_bf = bfp_t[:, 128:129]
            pw_at = {1: 129, 0: 129 + WIN, 8: 129 + 2 * WIN}  # t -> col
            C0 = 129 + 3 * WIN
            wf2_b = bfp_t[:, C0:C0 + 6]
            ones_row = bfp_t[0:1, C0 + 6:C0 + 6 + 128]
            bvoo_row = bfp_t[0:1, C0 + 134:C0 + 134 + H]
            bt_row = bfp_t[0:1, C0 + 134 + H:C0 + 134 + 3 * H]
            wf1_t = pers.tile([3, H + 1], BF16, tag="wf1")
            nc.gpsimd.dma_start(wf1_t[:], wf1p[:])
            ones11 = wf1_t[0:1, H:H + 1]
            wte_t = pers.tile([64, H + 1], BF16, tag="wte")
            nc.gpsimd.dma_start(wte_t[:], wtep[:])
            f8p_t = pers.tile([128, 8], F8, tag="f8p")
            nc.sync.dma_start(f8p_t[:], f8p[:])

            # ---------- persistent activations ----------
            xp8 = pers.tile([128, HC * NB * 128], F8, tag="xp8")
            kT8 = pers.tile([128, HC * NB * 128], F8, tag="kT8")
            qT8 = pers.tile([128, HC * 1024], F8, tag="qT8")
            h1T = pers.tile([128, 3 * 1024], F8, tag="h1T")
            h2T = pers.tile([128, 2 * 1024], F8, tag="h2T")
            fu1s = pers.tile([128, HC * 1024], BF16, tag="fu1s")
            vw = pers.tile([128, NB * H], BF16, tag="vw")
            bp8 = pers.tile([128, 8 * WIN], BF16, tag="bp8")
            rs8 = pers.tile([128, 8], F32, tag="rs8")
            ci8 = pers.tile([128, 8], F32, tag="ci8")
            fw8 = pers.tile([128, 8], F32, tag="fw8")
            g8 = pers.tile([128, 8], F32, tag="g8")
            zr8 = pers.tile([128, 8], F32, tag="zr8")
            ci_r = pers.tile([1, 1024], BF16, tag="cir")
            fwrow = pers.tile([1, 1024], BF16, tag="fwrow")
            ftin = pers.tile([128, 24], BF16, tag="ftin")
            fin = pers.tile([3, 1024], BF16, tag="fin")
            vswo_b = pers.tile([1, H], BF16, tag="vswo")
            ts_b = pers.tile([128, 1], F32, tag="tsb")
            ts1_b = pers.tile([128, 1], F32, tag="ts1b")
            vs_ps = pvs.tile([1, WIN], F32, tag="vsps")
            xp_all = pers.tile([128, NB * H], BF16, tag="xpall")

            x8v = r3(xp8[:], HC)
            k8v = r3(kT8[:], HC)
            q8v = r3(qT8[:], HC)
            h1v = r3(h1T[:], 3)
            h2v = r3(h2T[:], 2)
            fuv = r3(fu1s[:], HC)
            vwv = r3(vw[:], NB)

            # eviction engine rotation (DVE / Pool only where noted)
            def ev(idx, engs, dst, src, s1=None, bias=None, relu=False):
                e = engs[idx % len(engs)]
                if e is nc.scalar:
                    fn = AF.Relu if relu else AF.Identity
                    nc.scalar.activation(dst, src, fn,
                                         bias=(bias if bias is not None else 0.0),
                                         scale=(s1 if s1 is not None else 1.0))
                else:
                    if relu:
                        assert s1 is None
                        e.tensor_scalar(dst, src,
                                        bias if bias is not None else 0.0,
                                        0.0, ALU.add, ALU.max)
                    elif s1 is not None and bias is not None:
                        e.tensor_scalar(dst, src, s1, bias, ALU.mult, ALU.add)
                    elif s1 is not None:
                        e.tensor_scalar_mul(dst, src, s1)
                    elif bias is not None:
                        e.tensor_scalar_add(dst, src, bias)
                    else:
                        e.tensor_copy(dst, src)

            # ---------- phase X: load x/pos, add, transpose, cast ----------
            with (
                tc.tile_pool(name="xld", bufs=5) as xld,
                tc.tile_pool(name="xtp", bufs=1) as xtp,
            ):
                xpT16 = xtp.tile([128, HC * NB * 128], BF16, tag="xpT16")
                xTv = r3(xpT16[:], HC)
                cast_engs = [nc.vector, nc.scalar]
                wts = {}
                wdma = {0: ("wvo", wvo8, H, nc.sync),
                        1: ("wk", wk8, H, nc.scalar),
                        4: ("wq", wq8, H, nc.sync),
                        5: ("wc1", wc18, 384, nc.scalar)}
                wbdma = {8: ("wvob", wvob, H)}
                xsq = [nc.sync, nc.scalar, nc.gpsimd, nc.sync,
                       nc.scalar, nc.gpsimd, nc.sync, nc.scalar,
                       nc.gpsimd, nc.sync]
                for u in range(NB):
                    xs = xld.tile([128, 2 * H], BF16, tag="xs")
                    xsq[u].dma_start(xs[:], xp2[u * 128:(u + 1) * 128, :])
                    q_ = nc.sync if u % 2 == 0 else nc.scalar
                    if u in wdma:
                        nm, dram, wdt, wq_ = wdma[u]
                        wt = wp.tile([128, HC * wdt], F8, tag="w")
                        wq_.dma_start(
                            r3(wt[:], HC),
                            dram[:].rearrange("(c p) n -> p c n", c=HC))
                        wts[nm] = wt
                    if u in wbdma:
                        nm, dram, wdt = wbdma[u]
                        wt = wbp.tile([128, HC * wdt], BF16, tag="wb")
                        nc.sync.dma_start(
                            r3(wt[:], HC)[:, 0:3, :],
                            dram[0:384, :].rearrange("(c p) n -> p c n", c=3))
                        nc.scalar.dma_start(
                            r3(wt[:], HC)[:, 3:6, :],
                            dram[384:768, :].rearrange("(c p) n -> p c n", c=3))
                        wts[nm] = wt
                    xu = xp_all[:, u * H:(u + 1) * H]
                    nc.vector.tensor_tensor(xu, xs[:, 0:H], xs[:, H:2 * H],
                                            ALU.add)
                    q_.dma_start(xTv[:, :, u * 128:(u + 1) * 128], xu,
                                 transpose=True)
                    if u % 2 == 0:
                        nc.vector.tensor_copy(
                            x8v[:, :, u * 128:(u + 1) * 128],
                            xTv[:, :, u * 128:(u + 1) * 128])
                    else:
                        nc.scalar.activation(
                            x8v[:, :, u * 128:(u + 1) * 128],
                            xTv[:, :, u * 128:(u + 1) * 128], AF.Identity)
                wvo_t, wk_t = wts["wvo"], wts["wk"]
                wvob_t = wts["wvob"]

                # ---------- VW = xp @ WvoT (fp8 DR), colsum, collective ------
                wvo_v = r3(wvo_t[:], HC)
                for u in range(NB):
                    for n2 in range(2):
                        p = pB.tile([128, WIN], F32, tag="pb")
                        for n0, nn in ((0, 256), (256, 128)):
                            for c in range(3):
                                nc.tensor.matmul(
                                    p[:, n0:n0 + nn],
                                    x8v[:, 2 * c:2 * c + 2,
                                        u * 128:(u + 1) * 128],
                                    wvo_v[:, 2 * c:2 * c + 2,
                                          n2 * WIN + n0:n2 * WIN + n0 + nn],
                                    start=(c == 0), stop=(c == 2),
                                    perf_mode=DR,
                                    skip_group_check=True)
                        ev(u * 2 + n2, [nc.vector, nc.scalar],
                           vw[:, u * H + n2 * WIN:u * H + (n2 + 1) * WIN],
                           p[:], s1=1.0 / WVO_SC)
                vsrow = pers.tile([1, H], F32, tag="vsrow")
                for n2 in range(2):
                    for u in range(1, 9):
                        nc.tensor.matmul(
                            vs_ps[:],
                            ones_bf,
                            xp_all[:, u * H + n2 * WIN:u * H + (n2 + 1) * WIN],
                            start=(u == 1), stop=(u == 8),
                            skip_group_check=True)
                    nc.vector.tensor_copy(vsrow[:, n2 * WIN:(n2 + 1) * WIN],
                                          vs_ps[:])
                # ---------- K / Q / h1 projections (fp8 DR) ----------
                wq_t, wc1_t = wts["wq"], wts["wc1"]

                def proj(wv_, oc_n, j0, j1, out2d, outw, bias_col, ei):
                    jc = j0
                    while jc < j1:
                        jn = min(512, j1 - jc)
                        for oc in range(oc_n):
                            p = pA.tile([128, jn], F32, tag="pk")
                            n0 = 0
                            while n0 < jn:
                                nn = min(256, jn - n0)
                                for c in range(3):
                                    nc.tensor.matmul(
                                        p[:, n0:n0 + nn],
                                        wv_[:, 2 * c:2 * c + 2,
                                            oc * 128:(oc + 1) * 128],
                                        x8v[:, 2 * c:2 * c + 2,
                                            jc + n0:jc + n0 + nn],
                                        start=(c == 0), stop=(c == 2),
                                        perf_mode=DR,
                                        skip_group_check=True)
                                n0 += nn
                            ev(ei, [nc.vector, nc.scalar],
                               out2d[:, oc * outw + (jc - j0):
                                     oc * outw + (jc - j0) + jn],
                               p[:], bias=smf_t[:, bias_col + oc:bias_col + oc + 1])
                            ei += 1
                        jc += jn
                    return ei

                proj(r3(wk_t[:], HC), HC, 0, 1280, kT8, 1280, 6, 0)
                proj(r3(wq_t[:], HC), HC, 128, 1152, qT8, 1024, 0, 1)
                proj(r3(wc1_t[:], HC), 3, 128, 1152, h1T, 1024, 12, 0)

                with tc.tile_pool(name="ccd", bufs=1, space="DRAM") as ccd:
                    cc_in = ccd.tile([1, H], F32, tag="cci")
                    cc_out = ccd.tile([2, H], F32, tag="cco")
                    nc.gpsimd.dma_start(cc_in[:], vsrow[:])
                    nc.gpsimd.collective_compute(
                        "AllGather", ALU.bypass,
                        replica_groups=[[0, 1], [2, 3], [4, 5], [6, 7]],
                        ins=[cc_in[:].opt()], outs=[cc_out[:].opt()])
                    vsg2 = pers.tile([1, 2 * H], F32, tag="vsg2")
                    nc.sync.dma_start(vsg2[:], cc_out[:])

                # task MLP (rows; biases via rank-1)
                t1row = pers.tile([1, H], BF16, tag="t1row")
                for n0, nn in ((0, 512), (512, 256)):
                    p = pA.tile([1, nn], F32, tag="pk")
                    nc.tensor.matmul(p[:], wte_t[:, H:H + 1],
                                     wte_t[:, n0:n0 + nn],
                                     start=True, stop=False,
                                     skip_group_check=True)
                    nc.tensor.matmul(p[:], ones11, bt_row[:, n0:n0 + nn],
                                     start=False, stop=True,
                                     skip_group_check=True)
                    nc.scalar.activation(t1row[:, n0:n0 + nn], p[:], AF.Relu)
                t1c = pers.tile([128, HC], BF16, tag="t1c")
                for c in range(HC):
                    pt = pT.tile([128, 1], BF16, tag="pt")
                    nc.tensor.transpose(pt[:], t1row[:, c * 128:(c + 1) * 128],
                                        ones11)
                    nc.vector.tensor_copy(t1c[:, c:c + 1], pt[:])
                wt2_t = wbp.tile([128, HC * H], BF16, tag="w2")
                nc.sync.dma_start(
                    r3(wt2_t[:], HC)[:, 0:3, :],
                    wt2b[0:384, :].rearrange("(c p) n -> p c n", c=3))
                nc.scalar.dma_start(
                    r3(wt2_t[:], HC)[:, 3:6, :],
                    wt2b[384:768, :].rearrange("(c p) n -> p c n", c=3))
                wt2v = r3(wt2_t[:], HC)
                twrow = pers.tile([1, H], F32, tag="twrow")
                for n2 in range(2):
                    p = pA.tile([1, WIN], F32, tag="pk")
                    for c in range(HC):
                        nc.tensor.matmul(p[:], t1c[:, c:c + 1],
                                         wt2v[:, c, n2 * WIN:(n2 + 1) * WIN],
                                         start=(c == 0), stop=False,
                                         skip_group_check=True)
                    nc.tensor.matmul(p[:], ones11,
                                     bt_row[:, H + n2 * WIN:H + (n2 + 1) * WIN],
                                     start=False, stop=True,
                                     skip_group_check=True)
                    nc.scalar.activation(twrow[:, n2 * WIN:(n2 + 1) * WIN],
                                         p[:], AF.Sigmoid)
                tsum = pers.tile([1, 1], F32, tag="tsum")
                nc.vector.tensor_reduce(tsum[:], twrow[:], AX.X, ALU.add)
                ts_t = pers.tile([1, 1], F32, tag="tst")
                nc.vector.tensor_scalar_mul(ts_t[:], tsum[:], 1.0 / H)
                nc.gpsimd.partition_broadcast(ts_b[:], ts_t[:])
                nc.vector.tensor_scalar(ts1_b[:], ts_b[:], -0.5, 1.0,
                                        ALU.mult, ALU.add)

            # ---------- scores + bp + rowsums ----------
            for t in range(1, 9):
                p = pB.tile([128, WIN], F32, tag="pb")
                for n0, nn in ((0, 256), (256, 128)):
                    for c in range(3):
                        nc.tensor.matmul(
                            p[:, n0:n0 + nn],
                            q8v[:, 2 * c:2 * c + 2,
                                (t - 1) * 128:t * 128],
                            k8v[:, 2 * c:2 * c + 2,
                                (t - 1) * 128 + n0:(t - 1) * 128 + n0 + nn],
                            start=(c == 0), stop=(c == 2),
                            perf_mode=DR, skip_group_check=True)
                pwc = pw_at.get(t, pw_at[0])
                nc.vector.tensor_tensor(bp8[:, (t - 1) * WIN:t * WIN], p[:],
                                        bfp_t[:, pwc:pwc + WIN], ALU.mult)
                nc.vector.tensor_reduce(rs8[:, t - 1:t],
                                        bp8[:, (t - 1) * WIN:t * WIN],
                                        AX.X, ALU.add)

            # ---------- h2 / ci ----------
            wc2_t = pers.tile([128, 3 * 256], F8, tag="wc2")
            nc.sync.dma_start(wc2_t[:].rearrange("p (c n) -> p c n", c=3),
                              wc28[:].rearrange("(c p) n -> p c n", c=3))
            wc2v = wc2_t[:].rearrange("p (c n) -> p c n", c=3)
            for oc in range(2):
                for j2 in range(2):
                    p = pA.tile([128, 512], F32, tag="pk")
                    for c in range(3):
                        nc.tensor.matmul(
                            p[:], wc2v[:, c, oc * 128:(oc + 1) * 128],
                            h1v[:, c, j2 * 512:(j2 + 1) * 512],
                            start=(c == 0), stop=(c == 2),
                            skip_group_check=True)
                    nc.scalar.activation(
                        h2T[:, oc * 1024 + j2 * 512:oc * 1024 + (j2 + 1) * 512],
                        p[:], AF.Relu, bias=smf_t[:, 15 + oc:16 + oc])
            for j2 in range(2):
                p = pA.tile([1, 512], F32, tag="pk")
                for cc in range(2):
                    nc.tensor.matmul(p[:], f8p_t[:, 6 + cc:7 + cc],
                                     h2v[:, cc, j2 * 512:(j2 + 1) * 512],
                                     start=(cc == 0), stop=(cc == 1),
                                     skip_group_check=True)
                nc.scalar.activation(ci_r[:, j2 * 512:(j2 + 1) * 512], p[:],
                                     AF.Sigmoid, bias=smf_t[0:1, 23:24],
                                     scale=1.0 / WC3_SC)
            for t in range(8):
                pt = pT.tile([128, 1], BF16, tag="pt")
                nc.tensor.transpose(pt[:], ci_r[:, t * 128:(t + 1) * 128],
                                    ones11)
                nc.vector.tensor_copy(ci8[:, t:t + 1], pt[:])

            # ---------- fusion MLP (batched over all 1024 rows) ----------
            ftv = ftin[:].rearrange("p (t k) -> p t k", t=8)
            nc.vector.tensor_copy(ftv[:, :, 0], rs8[:])
            nc.vector.tensor_scalar_mul(ftv[:, :, 1], rs8[:], ts_b[:])
            nc.vector.tensor_tensor(ftv[:, :, 2], rs8[:], ci8[:], ALU.mult)
            for t in range(8):
                pf = pT.tile([3, 128], BF16, tag="pt")
                nc.tensor.transpose(pf[:], ftin[:, t * 3:(t + 1) * 3], identb)
                nc.vector.tensor_copy(fin[:, t * 128:(t + 1) * 128], pf[:])
            for oc in range(HC):
                for j2 in range(2):
                    p = pA.tile([128, 512], F32, tag="pk")
                    nc.tensor.matmul(p[:], wf1_t[0:3, oc * 128:(oc + 1) * 128],
                                     fin[:, j2 * 512:(j2 + 1) * 512],
                                     start=True, stop=True,
                                     skip_group_check=True)
                    ev(oc * 2 + j2, [nc.vector, nc.scalar],
                       fu1s[:, oc * 1024 + j2 * 512:oc * 1024 + (j2 + 1) * 512],
                       p[:], bias=smf_t[:, 17 + oc:18 + oc], relu=True)
            for j4 in range(4):
                p = pA.tile([1, 256], F32, tag="pk")
                for c in range(HC):
                    nc.tensor.matmul(p[:], wf2_b[:, c:c + 1],
                                     fuv[:, c, j4 * 256:(j4 + 1) * 256],
                                     start=(c == 0), stop=(c == HC - 1),
                                     skip_group_check=True)
                nc.scalar.activation(fwrow[:, j4 * 256:(j4 + 1) * 256], p[:],
                                     AF.Sigmoid, bias=smf_t[0:1, 24:25])
            for t in range(8):
                pt = pT.tile([128, 1], BF16, tag="pt")
                nc.tensor.transpose(pt[:], fwrow[:, t * 128:(t + 1) * 128],
                                    ones11)
                nc.vector.tensor_copy(fw8[:, t:t + 1], pt[:])
            pre8 = pers.tile([128, 8], F32, tag="pre8")
            nc.vector.tensor_scalar(pre8[:], ci8[:], -0.5, ts1_b[:],
                                    ALU.mult, ALU.add)
            nc.vector.tensor_tensor(g8[:], fw8[:], pre8[:], ALU.mult)
            nc.vector.tensor_scalar(g8[:], g8[:], -1.0, 1.0, ALU.mult, ALU.add)

            # far-field row: xsum_global -> column form -> @ Wvo (bf16)
            xsg = pers.tile([1, H], BF16, tag="xsg")
            nc.vector.tensor_tensor(xsg[:], vsg2[:, 0:H], vsg2[:, H:2 * H],
                                    ALU.add)
            xcol = pers.tile([128, HC], BF16, tag="xcol")
            for c in range(HC):
                pxc = pT.tile([128, 1], BF16, tag="pt")
                nc.tensor.transpose(pxc[:], xsg[:, c * 128:(c + 1) * 128],
                                    ones11)
                nc.vector.tensor_copy(xcol[:, c:c + 1], pxc[:])
            wvob_v = r3(wvob_t[:], HC)
            for n2 in range(2):
                pvs2 = pA.tile([1, WIN], F32, tag="pk")
                for c in range(HC):
                    nc.tensor.matmul(
                        pvs2[:], xcol[:, c:c + 1],
                        wvob_v[:, c, n2 * WIN:(n2 + 1) * WIN],
                        start=(c == 0), stop=(c == HC - 1),
                        skip_group_check=True)
                nc.vector.tensor_copy(vswo_b[:, n2 * WIN:(n2 + 1) * WIN],
                                      pvs2[:])

            # ---------- per-block tail: exp, E', y ----------
            with (
                tc.tile_pool(name="eb", bufs=8) as eb,
                tc.tile_pool(name="epb", bufs=8) as epb,
                tc.tile_pool(name="etp", bufs=8) as etp,
                tc.tile_pool(name="zrp", bufs=8) as zrp,
                tc.tile_pool(name="ytp", bufs=6) as ytp,
            ):
                for t in range(1, 9):
                    e_t = eb.tile([128, WIN], BF16, tag="et")
                    zc = zrp.tile([128, 1], F32, tag="zc")
                    nc.scalar.activation(e_t[:], bp8[:, (t - 1) * WIN:t * WIN],
                                         AF.Exp, scale=g8[:, t - 1:t],
                                         accum_out=zc[:])
                    nc.vector.tensor_scalar_add(zc[:], zc[:], float(S - WIN))
                    zrf = zrp.tile([128, 1], F32, tag="zrf")
                    nc.vector.reciprocal(zrf[:], zc[:])
                    ep = epb.tile([128, WIN], BF16, tag="ep")
                    nc.vector.tensor_scalar(ep[:], e_t[:], zrf[:],
                                            zrf[:], ALU.mult,
                                            ALU.subtract)
                    zrb = zrp.tile([128, 1], BF16, tag="zrb")
                    nc.vector.tensor_copy(zrb[:], zrf[:])
                    pz = pT.tile([1, 128], BF16, tag="pt")
                    nc.tensor.transpose(pz[:], zrb[:], identb)
                    zrow = zrp.tile([1, 128], BF16, tag="zrow")
                    nc.vector.tensor_copy(zrow[:], pz[:])
                    eT = etp.tile([128, 3 * 128], BF16, tag="eT")
                    for w in range(3):
                        pw_ = pT.tile([128, 128], BF16, tag="pt")
                        nc.tensor.transpose(
                            pw_[:], ep[:, w * 128:(w + 1) * 128], identb)
                        if (t + w) % 2 == 0:
                            nc.vector.tensor_copy(
                                eT[:, w * 128:(w + 1) * 128], pw_[:])
                        else:
                            nc.scalar.activation(
                                eT[:, w * 128:(w + 1) * 128], pw_[:],
                                AF.Identity)
                    eTv = r3(eT[:], 3)
                    y_t = ytp.tile([128, H], BF16, tag="yt")
                    for n2 in range(2):
                        p = pB.tile([128, WIN], F32, tag="pb")
                        for w in range(3):
                            nc.tensor.matmul(
                                p[:], eTv[:, w, :],
                                vwv[:, t - 1 + w, n2 * WIN:(n2 + 1) * WIN],
                                start=(w == 0), stop=False,
                                skip_group_check=True)
                        nc.tensor.matmul(p[:], zrow[:],
                                         vswo_b[:, n2 * WIN:(n2 + 1) * WIN],
                                         start=False, stop=False,
                                         skip_group_check=True)
                        nc.tensor.matmul(p[:], ones_row[:],
                                         bvoo_row[:, n2 * WIN:(n2 + 1) * WIN],
                                         start=False, stop=True,
                                         skip_group_check=True)
                        ev(t * 2 + n2, [nc.vector, nc.scalar],
                           y_t[:, n2 * WIN:(n2 + 1) * WIN], p[:])
                    nc.gpsimd.dma_start(y[(t - 1) * 128:t * 128, :], y_t[:])

                    if dbg:
                        nc.vector.tensor_copy(zr8[:, t - 1:t], zrf[:])
                if dbg:
                    twf = pers.tile([1, 1024], F32, tag="dtw")
                    nc.vector.tensor_copy(twf[:], ci_r[:])
                    nc.sync.dma_start(d_ci[:], twf[:])
                    fwf = pers.tile([1, 1024], F32, tag="dfw")
                    nc.vector.tensor_copy(fwf[:], fwrow[:])
                    nc.sync.dma_start(d_fw[:], fwf[:])
                    nc.sync.dma_start(d_ts[:], ts_t[:])
                    nc.sync.dma_start(d_g8[:], g8[:])
                    nc.sync.dma_start(d_zr8[:], zr8[:])
                    nc.sync.dma_start(d_rs8[:], rs8[:])
                    vsf = pers.tile([1, H], F32, tag="dvs")
                    nc.vector.tensor_copy(vsf[:], vswo_b[:])
                    nc.sync.dma_start(d_vswo[:], vsf[:])
                    ktf = pers.tile([128, 128], F32, tag="dkt")
                    nc.vector.tensor_copy(ktf[:], kT8[:, 128:256])
                    nc.sync.dma_start(d_kt[:], ktf[:])
                    qf = pers.tile([128, 128], F32, tag="dq")
                    nc.vector.tensor_copy(qf[:], qT8[:, 0:128])
                    nc.sync.dma_start(d_q[:], qf[:])
                    vwf = pers.tile([128, H], F32, tag="dvw")
                    nc.vector.tensor_copy(vwf[:], vw[:, H:2 * H])
                    nc.sync.dma_start(d_vw[:], vwf[:])
                    x8f = pers.tile([128, 256], F32, tag="dx8")
                    nc.vector.tensor_copy(x8f[:], xp8[:, 0:256])
                    nc.sync.dma_start(d_x8[:], x8f[:])
                    wkf = pers.tile([128, 256], F32, tag="dwk")
                    nc.vector.tensor_copy(wkf[:], wk_t[:, 0:256])
                    nc.sync.dma_start(d_wk[:], wkf[:])
                    bpf = pers.tile([128, 384], F32, tag="dbp")
                    nc.vector.tensor_copy(bpf[:], bp8[:, 0:384])
                    nc.sync.dma_start(d_bp[:], bpf[:])

    nc.compile()
    return nc


def prep_inputs(x, task_id, pos_emb, Wq, bq, Wk, bk, Wv, bv, Wo, bo,
                task_table, Wt1, bt1, Wt2, bt2,
                Wc1, bc1, Wc2, bc2, Wc3, bc3,
                Wf1, bf1, Wf2, bf2):
    f = np.float32
    bf = ml_dtypes.bfloat16
    f8 = mybir.dt.np(F8)
    T = lambda a: np.ascontiguousarray(np.asarray(a, dtype=f).T)
    c8 = lambda a: np.clip(np.asarray(a, dtype=f), -240, 240).astype(f8)

    wvo = T(Wv) @ T(Wo)                     # (Wo Wv)^T
    smf = np.zeros((128, 26), f)
    smf[:, 0:6] = np.asarray(bq).reshape(HC, 128).T
    smf[:, 6:12] = np.asarray(bk).reshape(HC, 128).T
    smf[:, 12:15] = np.asarray(bc1).reshape(3, 128).T
    bc2p = np.zeros(256, f); bc2p[:192] = np.asarray(bc2)
    smf[:, 15:17] = bc2p.reshape(2, 128).T
    smf[:, 17:23] = np.asarray(bf1).reshape(HC, 128).T
    smf[:, 23] = float(np.asarray(bc3).reshape(-1)[0])
    smf[:, 24] = float(np.asarray(bf2).reshape(-1)[0])

    p_ = np.arange(128)[:, None]; c_ = np.arange(WIN)[None, :]
    pwm = (np.exp(-np.abs(128 + p_ - c_) / 2.0) / math.sqrt(H)).astype(f)

    wc2p = np.zeros((384, 256), f); wc2p[:, :192] = T(Wc2)
    wc3p = np.zeros((256,), f); wc3p[:192] = np.asarray(Wc3, dtype=f).reshape(-1)
    f8pk = np.zeros((128, 8), f)
    f8pk[:, 6:8] = (wc3p.reshape(2, 128).T) * WC3_SC

    wtep = np.zeros((64, H + 1), bf)
    wtep[:, 0:H] = T(Wt1).astype(bf)
    wf1p = np.zeros((3, H + 1), bf)
    wf1p[:, 0:H] = (T(Wf1) / S).astype(bf)
    wf1p[0, H] = bf(1.0)
    bvoo = (np.asarray(bv, f) @ np.asarray(Wo, f).T + np.asarray(bo, f)).reshape(H)

    shared = {
        "wq8": c8(T(Wq)), "wk8": c8(T(Wk)), "wvo8": c8(wvo * WVO_SC),
        "wvob": wvo.astype(bf),
        "wc18": c8(T(Wc1)), "wc28": c8(wc2p),
        "wt2b": T(Wt2).astype(bf), "wtep": wtep, "wf1p": wf1p,
        "f8p": c8(f8pk),
        "smf": smf,
    }

    x = np.asarray(x); pos_emb = np.asarray(pos_emb)
    task_table = np.asarray(task_table); task_id = np.asarray(task_id)
    in_maps = []
    for core in range(8):
        b, half = core // 2, core % 2
        g0 = 1024 * half - 128
        xp2 = np.zeros((NB * 128, 2 * H), bf)
        lo, hi = max(0, g0), min(S, g0 + NB * 128)
        xp2[lo - g0:hi - g0, 0:H] = x[b, lo:hi]
        xp2[lo - g0:hi - g0, H:2 * H] = pos_emb[0, lo:hi]
        pwf = pwm.copy(); pwl = pwm.copy()
        if half == 0:
            pwf[:, :128] = 0
        if half == 1:
            pwl[:, 256:] = 0
        C0 = 129 + 3 * WIN
        bfpk = np.zeros((128, C0 + 134 + 3 * H), bf)
        bfpk[:, 0:128] = np.eye(128, dtype=f).astype(bf)
        bfpk[:, 128] = bf(1.0)
        bfpk[:, 129:129 + WIN] = pwf.astype(bf)
        bfpk[:, 129 + WIN:129 + 2 * WIN] = pwm.astype(bf)
        bfpk[:, 129 + 2 * WIN:C0] = pwl.astype(bf)
        bfpk[:, C0:C0 + 6] = np.asarray(Wf2, f).reshape(HC, 128).T.astype(bf)
        bfpk[0, C0 + 6:C0 + 134] = bf(1.0)
        bfpk[0, C0 + 134:C0 + 134 + H] = bvoo.astype(bf)
        bfpk[0, C0 + 134 + H:C0 + 134 + 2 * H] = np.asarray(bt1, f).astype(bf)
        bfpk[0, C0 + 134 + 2 * H:C0 + 134 + 3 * H] = np.asarray(bt2, f).astype(bf)
        wte = wtep.copy()
        wte[:, H] = task_table[int(task_id[b])].astype(bf)
        m = dict(shared)
        m.update({"xp2": xp2, "bfp": bfpk, "wtep": wte})
        in_maps.append(m)
    return in_maps


class _Runner:
    """Compile the SPMD graph once and keep a reusable jitted callable."""

    def __init__(self, nc, n_cores=8):
        import jax
        from jax.sharding import Mesh, PartitionSpec
        from jax.experimental.shard_map import shard_map
        from concourse import bass2jax, mybir as _mb
        bass2jax.install_neuronx_cc_hook()
        self.nc = nc
        partition_name = (nc.partition_id_tensor.name
                          if nc.partition_id_tensor else None)
        in_names, out_names, out_avals, zero_outs = [], [], [], []
        for alloc in nc.m.functions[0].allocations:
            if not isinstance(alloc, _mb.MemoryLocationSet):
                continue
            name = alloc.memorylocations[0].name
            if alloc.kind == "ExternalInput":
                if name != partition_name:
                    in_names.append(name)
            elif alloc.kind == "ExternalOutput":
                shape = tuple(alloc.tensor_shape)
                dtype = _mb.dt.np(alloc.dtype)
                out_names.append(name)
                out_avals.append(jax.core.ShapedArray(shape, dtype))
                zero_outs.append(np.zeros(shape, dtype))
        self.in_names = list(in_names)
        self.out_names = out_names
        self.out_avals = out_avals
        self.zero_outs = zero_outs
        self.n_cores = n_cores
        n_params = len(self.in_names)
        all_in = list(self.in_names) + list(out_names)
        if partition_name is not None:
            all_in.append(partition_name)

        def _body(*args):
            operands = list(args)
            if partition_name is not None:
                operands.append(bass2jax.partition_id_tensor())
            outs = bass2jax._bass_exec_p.bind(
                *operands,
                out_avals=tuple(out_avals),
                in_names=tuple(all_in),
                out_names=tuple(out_names),
                lowering_input_output_aliases=(),
                sim_require_finite=True,
                sim_require_nnan=True,
                nc=nc,
            )
            return tuple(outs)

        devices = jax.devices()[:n_cores]
        mesh = Mesh(np.asarray(devices), ("core",))
        n_outs = len(out_names)
        in_specs = (PartitionSpec("core"),) * (n_params + n_outs)
        out_specs = (PartitionSpec("core"),) * n_outs
        self.fn = jax.jit(
            shard_map(_body, mesh=mesh, in_specs=in_specs,
                      out_specs=out_specs, check_rep=False),
            keep_unused=True)

    def concat_inputs(self, in_maps):
        return [np.concatenate([np.asarray(in_maps[c][k])
                                for c in range(self.n_cores)], axis=0)
                for k in self.in_names]

    def zeros(self):
        return [np.zeros((self.n_cores * z.shape[0],) + z.shape[1:], z.dtype)
                for z in self.zero_outs]

    def __call__(self, concat_in, zeros=None):
        import jax
        if zeros is None:
            zeros = self.zeros()
        outs = jax.block_until_ready(self.fn(*concat_in, *zeros))
        return outs


def get_runner(dbg=False):
    key = "runner_dbg" if dbg else "runner"
    if key not in _cache:
        _cache[key] = _Runner(build_kernel(dbg=dbg))
    return _cache[key]


def kernel(**inputs):
    rn = get_runner()
    in_maps = prep_inputs(**inputs)
    outs = rn(rn.concat_inputs(in_maps))
    yc = np.asarray(outs[rn.out_names.index("y")]).reshape(8, 1024, H)
    out = np.zeros((B, S, H), np.float32)
    for core in range(8):
        b, half = core // 2, core % 2
        out[b, 1024 * half:1024 * (half + 1)] = yc[core]
    return out


# revision 37
# speedup vs baseline: 1.0196x; 1.0196x over previous
"""Trainium2 Bass kernel for AdaptivePositionAwareAttention.

Banded softmax + closed-form far field (see reference): pos_w = exp(-|i-j|/2)
kills exp(f)-1 beyond ~90 positions, so per 128-row query block t only key
blocks t-1..t+1 (WIN=384) matter:

    y_i = zr_i * (VsumWo + sum_band (e^{f_ij}-1) VW_j) + (bv@Wo + bo)

where VW = xp @ (Wo Wv)^T (Wo folded into Wv on host), VsumWo = column-sum of
VW over the full sequence (cross-half part via one AllGather), and
zr_i = 1/(S - WIN + sum_band e^f).  The task/content/fusion branches reduce to
a per-row scalar g with f = base * g.

Layout/dtype scheme: x+pos loaded fp16, added on DVE, transposed feature-major
by the DMA XBAR (one 3D-AP DmaTranspose per block), cast to fp8.  The heavy
projections (K, Q, h1, VW) and scores run as fp8 DoubleRow matmuls (2 k-planes
per pass via 3D APs); E'@VW runs bf16.  zr is folded into E' so the output
needs only one vector add per block.

Sharding: 8 cores = (batch 0..3) x (sequence half); each core owns 1024 query
rows plus one 128-row zero-padded halo block per side.  Sequence-edge effects
ride on per-core masked pos_w *data*, so all cores run one SPMD graph.
"""

import math

import numpy as np
import ml_dtypes

import concourse.bass as bass
from concourse import bacc, mybir, tile
from concourse.bass_utils import run_bass_kernel_spmd

B, S, H = 4, 2048, 768
HC = H // 128             # 6 feature chunks
NB = 10                   # halo blocks per core (1280 rows)
WIN = 384                 # key window = 3 blocks
F32 = mybir.dt.float32
F16 = mybir.dt.float16
BF16 = mybir.dt.bfloat16
F8 = mybir.dt.float8e4
AF = mybir.ActivationFunctionType
ALU = mybir.AluOpType
AX = mybir.AxisListType
DR = mybir.MatmulPerfMode.DoubleRow

WVO_SC = 16.0             # host scale on (Wo Wv)^T to dodge fp8 subnormals
WC3_SC = 64.0
FU1_SC = 4096.0           # fu1s eviction scale (values ~3e-5 raw)
WF2_SC = 64.0

_cache = {}


def r3(ap, c):
    return ap.rearrange("p (c s) -> p c s", c=c)


def build_kernel(dbg=False):
    nc = bacc.Bacc(None, target_bir_lowering=False)

    def din(name, shape, dt=F32):
        return nc.dram_tensor(name, shape, dt, kind="ExternalInput")

    xp2 = din("xp2", [NB * 128, 2 * H], BF16)   # rows: [x | pos], zero-padded
    wq8 = din("wq8", [H, H], F8)                # Wq^T  [hi, ho]
    wk8 = din("wk8", [H, H], F8)
    wvo8 = din("wvo8", [H, H], F8)              # (Wo Wv)^T * WVO_SC
    wvob = din("wvob", [H, H], BF16)            # (Wo Wv)^T exact-ish
    wc18 = din("wc18", [H, 384], F8)            # Wc1^T
    wc28 = din("wc28", [384, 256], F8)          # Wc2^T zero-padded 192->256
    wt2b = din("wt2b", [H, H], BF16)            # Wt2^T
    wtep = din("wtep", [64, H + 1], BF16)       # [Wt1^T | te]
    wf1p = din("wf1p", [3, H + 1], BF16)        # [Wf1^T / S | ones col]
    f8p = din("f8p", [128, 8], F8)              # cols 0-5 Wf2*64, 6-7 Wc3*64
    smf = din("smf", [128, 26], F32)            # packed per-partition consts
    # smf cols: 0-5 bq, 6-11 bk, 12-14 bc1, 15-16 bc2, 17-22 bf1*FU1_SC,
    #           23 bc3 (bcast), 24 bf2 (bcast), 25 unused
    # bfp cols: 0:128 ident, 128 ones col, 129+ 3*WIN pw f/m/l, then wf2 [128,6],
    # then row-0-only: ones_row[128], bvoo[768], bt1|bt2[1536]
    NBF = 129 + 3 * WIN + 6 + 128 + H + 2 * H
    bfp = din("bfp", [128, NBF], BF16)

    y = nc.dram_tensor("y", [1024, H], BF16, kind="ExternalOutput")
    if dbg:
        d_ts = nc.dram_tensor("d_ts", [1, 1], F32, kind="ExternalOutput")
        d_ci = nc.dram_tensor("d_ci", [1, 1024], F32, kind="ExternalOutput")
        d_g8 = nc.dram_tensor("d_g8", [128, 8], F32, kind="ExternalOutput")
        d_zr8 = nc.dram_tensor("d_zr8", [128, 8], F32, kind="ExternalOutput")
        d_vswo = nc.dram_tensor("d_vswo", [1, H], F32, kind="ExternalOutput")
        d_rs8 = nc.dram_tensor("d_rs8", [128, 8], F32, kind="ExternalOutput")
        d_fw = nc.dram_tensor("d_fw", [1, 1024], F32, kind="ExternalOutput")
        d_kt = nc.dram_tensor("d_kt", [128, 128], F32, kind="ExternalOutput")
        d_vw = nc.dram_tensor("d_vw", [128, H], F32, kind="ExternalOutput")
        d_q = nc.dram_tensor("d_q", [128, 128], F32, kind="ExternalOutput")
        d_x8 = nc.dram_tensor("d_x8", [128, 256], F32, kind="ExternalOutput")
        d_wk = nc.dram_tensor("d_wk", [128, 256], F32, kind="ExternalOutput")
        d_bp = nc.dram_tensor("d_bp", [128, 384], F32, kind="ExternalOutput")

    with tile.TileContext(nc) as tc:
        with (
            tc.tile_pool(name="pers", bufs=1) as pers,
            tc.tile_pool(name="wp", bufs=4) as wp,
            tc.tile_pool(name="wbp", bufs=1) as wbp,
            tc.tile_pool(name="pA", bufs=3, space="PSUM") as pA,
            tc.tile_pool(name="pT", bufs=2, space="PSUM") as pT,
            tc.tile_pool(name="pB", bufs=2, space="PSUM") as pB,
            tc.tile_pool(name="pvs", bufs=1, space="PSUM") as pvs,
        ):
            # ---------- small packed loads ----------
            smf_t = pers.tile([128, 26], F32, tag="smf")
            nc.sync.dma_start(smf_t[:], smf[:])
            bfp_t = pers.tile([128, NBF], BF16, tag="bfp")
            nc.scalar.dma_start(bfp_t[:], bfp[:])
            identb = bfp_t[:, 0:128]
            ones_bf = bfp_t[:, 128:129]
            pw_at = {1: 129, 0: 129 + WIN, 8: 129 + 2 * WIN}  # t -> col
            C0 = 129 + 3 * WIN
            wf2_b = bfp_t[:, C0:C0 + 6]
            ones_row = bfp_t[0:1, C0 + 6:C0 + 6 + 128]
            bvoo_row = bfp_t[0:1, C0 + 134:C0 + 134 + H]
            bt_row = bfp_t[0:1, C0 + 134 + H:C0 + 134 + 3 * H]
            wf1_t = pers.tile([3, H + 1], BF16, tag="wf1")
            nc.gpsimd.dma_start(wf1_t[:], wf1p[:])
            ones11 = wf1_t[0:1, H:H + 1]
            wte_t = pers.tile([64, H + 1], BF16, tag="wte")
            nc.gpsimd.dma_start(wte_t[:], wtep[:])
            f8p_t = pers.tile([128, 8], F8, tag="f8p")
            nc.sync.dma_start(f8p_t[:], f8p[:])

            # ---------- persistent activations ----------
            xp8 = pers.tile([128, HC * NB * 128], F8, tag="xp8")
            kT8 = pers.tile([128, HC * NB * 128], F8, tag="kT8")
            qT8 = pers.tile([128, HC * 1024], F8, tag="qT8")
            h1T = pers.tile([128, 3 * 1024], F8, tag="h1T")
            h2T = pers.tile([128, 2 * 1024], F8, tag="h2T")
            fu1s = pers.tile([128, HC * 1024], BF16, tag="fu1s")
            vw = pers.tile([128, NB * H], BF16, tag="vw")
            bp8 = pers.tile([128, 8 * WIN], BF16, tag="bp8")
            rs8 = pers.tile([128, 8], F32, tag="rs8")
            ci8 = pers.tile([128, 8], F32, tag="ci8")
            fw8 = pers.tile([128, 8], F32, tag="fw8")
            g8 = pers.tile([128, 8], F32, tag="g8")
            zr8 = pers.tile([128, 8], F32, tag="zr8")
            ci_r = pers.tile([1, 1024], BF16, tag="cir")
            fwrow = pers.tile([1, 1024], BF16, tag="fwrow")
            ftin = pers.tile([128, 24], BF16, tag="ftin")
            fin = pers.tile([3, 1024], BF16, tag="fin")
            vswo_b = pers.tile([1, H], BF16, tag="vswo")
            ts_b = pers.tile([128, 1], F32, tag="tsb")
            ts1_b = pers.tile([128, 1], F32, tag="ts1b")
            vs_ps = pvs.tile([1, WIN], F32, tag="vsps")
            xp_all = pers.tile([128, NB * H], BF16, tag="xpall")

            x8v = r3(xp8[:], HC)
            k8v = r3(kT8[:], HC)
            q8v = r3(qT8[:], HC)
            h1v = r3(h1T[:], 3)
            h2v = r3(h2T[:], 2)
            fuv = r3(fu1s[:], HC)
            vwv = r3(vw[:], NB)

            # eviction engine rotation (DVE / Pool only where noted)
            def ev(idx, engs, dst, src, s1=None, bias=None, relu=False):
                e = engs[idx % len(engs)]
                if e is nc.scalar:
                    fn = AF.Relu if relu else AF.Identity
                    nc.scalar.activation(dst, src, fn,
                                         bias=(bias if bias is not None else 0.0),
                                         scale=(s1 if s1 is not None else 1.0))
                else:
                    if relu:
                        assert s1 is None
                        e.tensor_scalar(dst, src,
                                        bias if bias is not None else 0.0,
                                        0.0, ALU.add, ALU.max)
                    elif s1 is not None and bias is not None:
                        e.tensor_scalar(dst, src, s1, bias, ALU.mult, ALU.add)
                    elif s1 is not None:
                        e.tensor_scalar_mul(dst, src, s1)
                    elif bias is not None:
                        e.tensor_scalar_add(dst, src, bias)
                    else:
                        e.tensor_copy(dst, src)

            # ---------- phase X: load x/pos, add, transpose, cast ----------
            with (
                tc.tile_pool(name="xld", bufs=6) as xld,
                tc.tile_pool(name="xtp", bufs=1) as xtp,
            ):
                xpT16 = xtp.tile([128, HC * NB * 128], BF16, tag="xpT16")
                xTv = r3(xpT16[:], HC)
                cast_engs = [nc.vector, nc.scalar]
                wts = {}
                wdma = {0: ("wvo", wvo8, H, nc.sync),
                        1: ("wk", wk8, H, nc.scalar),
                        4: ("wq", wq8, H, nc.sync),
                        5: ("wc1", wc18, 384, nc.scalar)}
                wbdma = {8: ("wvob", wvob, H)}
                xsq = [nc.sync, nc.scalar, nc.gpsimd, nc.sync,
                       nc.scalar, nc.gpsimd, nc.sync, nc.scalar,
                       nc.gpsimd, nc.sync]
                for u in range(NB):
                    xs = xld.tile([128, 2 * H], BF16, tag="xs")
                    xsq[u].dma_start(xs[:], xp2[u * 128:(u + 1) * 128, :])
                    q_ = nc.sync if u % 2 == 0 else nc.scalar
                    if u in wdma:
                        nm, dram, wdt, wq_ = wdma[u]
                        wt = wp.tile([128, HC * wdt], F8, tag="w")
                        wq_.dma_start(
                            r3(wt[:], HC),
                            dram[:].rearrange("(c p) n -> p c n", c=HC))
                        wts[nm] = wt
                    if u in wbdma:
                        nm, dram, wdt = wbdma[u]
                        wt = wbp.tile([128, HC * wdt], BF16, tag="wb")
                        nc.sync.dma_start(
                            r3(wt[:], HC)[:, 0:3, :],
                            dram[0:384, :].rearrange("(c p) n -> p c n", c=3))
                        nc.scalar.dma_start(
                            r3(wt[:], HC)[:, 3:6, :],
                            dram[384:768, :].rearrange("(c p) n -> p c n", c=3))
                        wts[nm] = wt
                    xu = xp_all[:, u * H:(u + 1) * H]
                    nc.vector.tensor_tensor(xu, xs[:, 0:H], xs[:, H:2 * H],
                                            ALU.add)
                    q_.dma_start(xTv[:, :, u * 128:(u + 1) * 128], xu,
                                 transpose=True)
                    if u % 2 == 0:
                        nc.vector.tensor_copy(
                            x8v[:, :, u * 128:(u + 1) * 128],
                            xTv[:, :, u * 128:(u + 1) * 128])
                    else:
                        nc.scalar.activation(
                            x8v[:, :, u * 128:(u + 1) * 128],
                            xTv[:, :, u * 128:(u + 1) * 128], AF.Identity)
                wvo_t, wk_t = wts["wvo"], wts["wk"]
                wvob_t = wts["wvob"]

                # ---------- VW = xp @ WvoT (fp8 DR), colsum, collective ------
                wvo_v = r3(wvo_t[:], HC)
                for u in range(NB):
                    for n2 in range(2):
                        p = pB.tile([128, WIN], F32, tag="pb")
                        for n0, nn in ((0, 256), (256, 128)):
                            for c in range(3):
                                nc.tensor.matmul(
                                    p[:, n0:n0 + nn],
                                    x8v[:, 2 * c:2 * c + 2,
                                        u * 128:(u + 1) * 128],
                                    wvo_v[:, 2 * c:2 * c + 2,
                                          n2 * WIN + n0:n2 * WIN + n0 + nn],
                                    start=(c == 0), stop=(c == 2),
                                    perf_mode=DR,
                                    skip_group_check=True)
                        ev(u * 2 + n2, [nc.vector, nc.scalar],
                           vw[:, u * H + n2 * WIN:u * H + (n2 + 1) * WIN],
                           p[:], s1=1.0 / WVO_SC)
                vsrow = pers.tile([1, H], F32, tag="vsrow")
                for n2 in range(2):
                    for u in range(1, 9):
                        nc.tensor.matmul(
                            vs_ps[:],
                            ones_bf,
                            xp_all[:, u * H + n2 * WIN:u * H + (n2 + 1) * WIN],
                            start=(u == 1), stop=(u == 8),
                            skip_group_check=True)
                    nc.vector.tensor_copy(vsrow[:, n2 * WIN:(n2 + 1) * WIN],
                                          vs_ps[:])
                # ---------- K / Q / h1 projections (fp8 DR) ----------
                wq_t, wc1_t = wts["wq"], wts["wc1"]

                def proj(wv_, oc_n, j0, j1, out2d, outw, bias_col, ei):
                    jc = j0
                    while jc < j1:
                        jn = min(512, j1 - jc)
                        for oc in range(oc_n):
                            p = pA.tile([128, jn], F32, tag="pk")
                            n0 = 0
                            while n0 < jn:
                                nn = min(256, jn - n0)
                                for c in range(3):
                                    nc.tensor.matmul(
                                        p[:, n0:n0 + nn],
                                        wv_[:, 2 * c:2 * c + 2,
                                            oc * 128:(oc + 1) * 128],
                                        x8v[:, 2 * c:2 * c + 2,
                                            jc + n0:jc + n0 + nn],
                                        start=(c == 0), stop=(c == 2),
                                        perf_mode=DR,
                                        skip_group_check=True)
                                n0 += nn
                            ev(ei, [nc.vector, nc.scalar],
                               out2d[:, oc * outw + (jc - j0):
                                     oc * outw + (jc - j0) + jn],
                               p[:], bias=smf_t[:, bias_col + oc:bias_col + oc + 1])
                            ei += 1
                        jc += jn
                    return ei

                proj(r3(wk_t[:], HC), HC, 0, 1280, kT8, 1280, 6, 0)
                proj(r3(wq_t[:], HC), HC, 128, 1152, qT8, 1024, 0, 1)
                proj(r3(wc1_t[:], HC), 3, 128, 1152, h1T, 1024, 12, 0)

                with tc.tile_pool(name="ccd", bufs=1, space="DRAM") as ccd:
                    cc_in = ccd.tile([1, H], F32, tag="cci")
                    cc_out = ccd.tile([2, H], F32, tag="cco")
                    nc.gpsimd.dma_start(cc_in[:], vsrow[:])
                    nc.gpsimd.collective_compute(
                        "AllGather", ALU.bypass,
                        replica_groups=[[0, 1], [2, 3], [4, 5], [6, 7]],
                        ins=[cc_in[:].opt()], outs=[cc_out[:].opt()])
                    vsg2 = pers.tile([1, 2 * H], F32, tag="vsg2")
                    nc.sync.dma_start(vsg2[:], cc_out[:])

                # task MLP (rows; biases via rank-1)
                t1row = pers.tile([1, H], BF16, tag="t1row")
                for n0, nn in ((0, 512), (512, 256)):
                    p = pA.tile([1, nn], F32, tag="pk")
                    nc.tensor.matmul(p[:], wte_t[:, H:H + 1],
                                     wte_t[:, n0:n0 + nn],
                                     start=True, stop=False,
                                     skip_group_check=True)
                    nc.tensor.matmul(p[:], ones11, bt_row[:, n0:n0 + nn],
                                     start=False, stop=True,
                                     skip_group_check=True)
                    nc.scalar.activation(t1row[:, n0:n0 + nn], p[:], AF.Relu)
                t1c = pers.tile([128, HC], BF16, tag="t1c")
                for c in range(HC):
                    pt = pT.tile([128, 1], BF16, tag="pt")
                    nc.tensor.transpose(pt[:], t1row[:, c * 128:(c + 1) * 128],
                                        ones11)
                    nc.vector.tensor_copy(t1c[:, c:c + 1], pt[:])
                wt2_t = wbp.tile([128, HC * H], BF16, tag="w2")
                nc.sync.dma_start(
                    r3(wt2_t[:], HC)[:, 0:3, :],
                    wt2b[0:384, :].rearrange("(c p) n -> p c n", c=3))
                nc.scalar.dma_start(
                    r3(wt2_t[:], HC)[:, 3:6, :],
                    wt2b[384:768, :].rearrange("(c p) n -> p c n", c=3))
                wt2v = r3(wt2_t[:], HC)
                twrow = pers.tile([1, H], F32, tag="twrow")
                for n2 in range(2):
                    p = pA.tile([1, WIN], F32, tag="pk")
                    for c in range(HC):
                        nc.tensor.matmul(p[:], t1c[:, c:c + 1],
                                         wt2v[:, c, n2 * WIN:(n2 + 1) * WIN],
                                         start=(c == 0), stop=False,
                                         skip_group_check=True)
                    nc.tensor.matmul(p[:], ones11,
                                     bt_row[:, H + n2 * WIN:H + (n2 + 1) * WIN],
                                     start=False, stop=True,
                                     skip_group_check=True)
                    nc.scalar.activation(twrow[:, n2 * WIN:(n2 + 1) * WIN],
                                         p[:], AF.Sigmoid)
                tsum = pers.tile([1, 1], F32, tag="tsum")
                nc.vector.tensor_reduce(tsum[:], twrow[:], AX.X, ALU.add)
                ts_t = pers.tile([1, 1], F32, tag="tst")
                nc.vector.tensor_scalar_mul(ts_t[:], tsum[:], 1.0 / H)
                nc.gpsimd.partition_broadcast(ts_b[:], ts_t[:])
                nc.vector.tensor_scalar(ts1_b[:], ts_b[:], -0.5, 1.0,
                                        ALU.mult, ALU.add)

            # ---------- scores + bp + rowsums ----------
            for t in range(1, 9):
                p = pB.tile([128, WIN], F32, tag="pb")
                for n0, nn in ((0, 256), (256, 128)):
                    for c in range(3):
                        nc.tensor.matmul(
                            p[:, n0:n0 + nn],
                            q8v[:, 2 * c:2 * c + 2,
                                (t - 1) * 128:t * 128],
                            k8v[:, 2 * c:2 * c + 2,
                                (t - 1) * 128 + n0:(t - 1) * 128 + n0 + nn],
                            start=(c == 0), stop=(c == 2),
                            perf_mode=DR, skip_group_check=True)
                pwc = pw_at.get(t, pw_at[0])
                nc.vector.tensor_tensor(bp8[:, (t - 1) * WIN:t * WIN], p[:],
                                        bfp_t[:, pwc:pwc + WIN], ALU.mult)
                nc.vector.tensor_reduce(rs8[:, t - 1:t],
                                        bp8[:, (t - 1) * WIN:t * WIN],
                                        AX.X, ALU.add)

            # ---------- h2 / ci ----------
            wc2_t = pers.tile([128, 3 * 256], F8, tag="wc2")
            nc.sync.dma_start(wc2_t[:].rearrange("p (c n) -> p c n", c=3),
                              wc28[:].rearrange("(c p) n -> p c n", c=3))
            wc2v = wc2_t[:].rearrange("p (c n) -> p c n", c=3)
            for oc in range(2):
                for j2 in range(2):
                    p = pA.tile([128, 512], F32, tag="pk")
                    for c in range(3):
                        nc.tensor.matmul(
                            p[:], wc2v[:, c, oc * 128:(oc + 1) * 128],
                            h1v[:, c, j2 * 512:(j2 + 1) * 512],
                            start=(c == 0), stop=(c == 2),
                            skip_group_check=True)
                    nc.scalar.activation(
                        h2T[:, oc * 1024 + j2 * 512:oc * 1024 + (j2 + 1) * 512],
                        p[:], AF.Relu, bias=smf_t[:, 15 + oc:16 + oc])
            for j2 in range(2):
                p = pA.tile([1, 512], F32, tag="pk")
                for cc in range(2):
                    nc.tensor.matmul(p[:], f8p_t[:, 6 + cc:7 + cc],
                                     h2v[:, cc, j2 * 512:(j2 + 1) * 512],
                                     start=(cc == 0), stop=(cc == 1),
                                     skip_group_check=True)
                nc.scalar.activation(ci_r[:, j2 * 512:(j2 + 1) * 512], p[:],
                                     AF.Sigmoid, bias=smf_t[0:1, 23:24],
                                     scale=1.0 / WC3_SC)
            for t in range(8):
                pt = pT.tile([128, 1], BF16, tag="pt")
                nc.tensor.transpose(pt[:], ci_r[:, t * 128:(t + 1) * 128],
                                    ones11)
                nc.vector.tensor_copy(ci8[:, t:t + 1], pt[:])

            # ---------- fusion MLP (batched over all 1024 rows) ----------
            ftv = ftin[:].rearrange("p (t k) -> p t k", t=8)
            nc.vector.tensor_copy(ftv[:, :, 0], rs8[:])
            nc.vector.tensor_scalar_mul(ftv[:, :, 1], rs8[:], ts_b[:])
            nc.vector.tensor_tensor(ftv[:, :, 2], rs8[:], ci8[:], ALU.mult)
            for t in range(8):
                pf = pT.tile([3, 128], BF16, tag="pt")
                nc.tensor.transpose(pf[:], ftin[:, t * 3:(t + 1) * 3], identb)
                nc.vector.tensor_copy(fin[:, t * 128:(t + 1) * 128], pf[:])
            for oc in range(HC):
                for j2 in range(2):
                    p = pA.tile([128, 512], F32, tag="pk")
                    nc.tensor.matmul(p[:], wf1_t[0:3, oc * 128:(oc + 1) * 128],
                                     fin[:, j2 * 512:(j2 + 1) * 512],
                                     start=True, stop=True,
                                     skip_group_check=True)
                    ev(oc * 2 + j2, [nc.vector, nc.scalar],
                       fu1s[:, oc * 1024 + j2 * 512:oc * 1024 + (j2 + 1) * 512],
                       p[:], bias=smf_t[:, 17 + oc:18 + oc], relu=True)
            for j4 in range(4):
                p = pA.tile([1, 256], F32, tag="pk")
                for c in range(HC):
                    nc.tensor.matmul(p[:], wf2_b[:, c:c + 1],
                                     fuv[:, c, j4 * 256:(j4 + 1) * 256],
                                     start=(c == 0), stop=(c == HC - 1),
                                     skip_group_check=True)
                nc.scalar.activation(fwrow[:, j4 * 256:(j4 + 1) * 256], p[:],
                                     AF.Sigmoid, bias=smf_t[0:1, 24:25])
            for t in range(8):
                pt = pT.tile([128, 1], BF16, tag="pt")
                nc.tensor.transpose(pt[:], fwrow[:, t * 128:(t + 1) * 128],
                                    ones11)
                nc.vector.tensor_copy(fw8[:, t:t + 1], pt[:])
            pre8 = pers.tile([128, 8], F32, tag="pre8")
            nc.vector.tensor_scalar(pre8[:], ci8[:], -0.5, ts1_b[:],
                                    ALU.mult, ALU.add)
            nc.vector.tensor_tensor(g8[:], fw8[:], pre8[:], ALU.mult)
            nc.vector.tensor_scalar(g8[:], g8[:], -1.0, 1.0, ALU.mult, ALU.add)

            # far-field row: xsum_global -> column form -> @ Wvo (bf16)
            xsg = pers.tile([1, H], BF16, tag="xsg")
            nc.vector.tensor_tensor(xsg[:], vsg2[:, 0:H], vsg2[:, H:2 * H],
                                    ALU.add)
            xcol = pers.tile([128, HC], BF16, tag="xcol")
            for c in range(HC):
                pxc = pT.tile([128, 1], BF16, tag="pt")
                nc.tensor.transpose(pxc[:], xsg[:, c * 128:(c + 1) * 128],
                                    ones11)
                nc.vector.tensor_copy(xcol[:, c:c + 1], pxc[:])
            wvob_v = r3(wvob_t[:], HC)
            for n2 in range(2):
                pvs2 = pA.tile([1, WIN], F32, tag="pk")
                for c in range(HC):
                    nc.tensor.matmul(
                        pvs2[:], xcol[:, c:c + 1],
                        wvob_v[:, c, n2 * WIN:(n2 + 1) * WIN],
                        start=(c == 0), stop=(c == HC - 1),
                        skip_group_check=True)
                nc.vector.tensor_copy(vswo_b[:, n2 * WIN:(n2 + 1) * WIN],
                                      pvs2[:])

            # ---------- per-block tail: exp, E', y ----------
            with (
                tc.tile_pool(name="eb", bufs=8) as eb,
                tc.tile_pool(name="epb", bufs=8) as epb,
                tc.tile_pool(name="etp", bufs=8) as etp,
                tc.tile_pool(name="zrp", bufs=8) as zrp,
                tc.tile_pool(name="ytp", bufs=8) as ytp,
            ):
                for t in range(1, 9):
                    e_t = eb.tile([128, WIN], BF16, tag="et")
                    zc = zrp.tile([128, 1], F32, tag="zc")
                    nc.scalar.activation(e_t[:], bp8[:, (t - 1) * WIN:t * WIN],
                                         AF.Exp, scale=g8[:, t - 1:t],
                                         accum_out=zc[:])
                    nc.vector.tensor_scalar_add(zc[:], zc[:], float(S - WIN))
                    zrf = zrp.tile([128, 1], F32, tag="zrf")
                    nc.vector.reciprocal(zrf[:], zc[:])
                    ep = epb.tile([128, WIN], BF16, tag="ep")
                    nc.vector.tensor_scalar(ep[:], e_t[:], zrf[:],
                                            zrf[:], ALU.mult,
                                            ALU.subtract)
                    zrb = zrp.tile([128, 1], BF16, tag="zrb")
                    nc.vector.tensor_copy(zrb[:], zrf[:])
                    pz = pT.tile([1, 128], BF16, tag="pt")
                    nc.tensor.transpose(pz[:], zrb[:], identb)
                    zrow = zrp.tile([1, 128], BF16, tag="zrow")
                    nc.vector.tensor_copy(zrow[:], pz[:])
                    eT = etp.tile([128, 3 * 128], BF16, tag="eT")
                    for w in range(3):
                        pw_ = pT.tile([128, 128], BF16, tag="pt")
                        nc.tensor.transpose(
                            pw_[:], ep[:, w * 128:(w + 1) * 128], identb)
                        if (t + w) % 2 == 0:
                            nc.vector.tensor_copy(
                                eT[:, w * 128:(w + 1) * 128], pw_[:])
                        else:
                            nc.scalar.activation(
                                eT[:, w * 128:(w + 1) * 128], pw_[:],
                                AF.Identity)
                    eTv = r3(eT[:], 3)
                    y_t = ytp.tile([128, H], BF16, tag="yt")
                    for n2 in range(2):
                        p = pB.tile([128, WIN], F32, tag="pb")
                        for w in range(3):
                            nc.tensor.matmul(
                                p[:], eTv[:, w, :],
                                vwv[:, t - 1 + w, n2 * WIN:(n2 + 1) * WIN],
                                start=(w == 0), stop=False,
                                skip_group_check=True)
                        nc.tensor.matmul(p[:], zrow[:],
                                         vswo_b[:, n2 * WIN:(n2 + 1) * WIN],
                                         start=False, stop=False,
                                         skip_group_check=True)
                        nc.tensor.matmul(p[:], ones_row[:],
                                         bvoo_row[:, n2 * WIN:(n2 + 1) * WIN],
                                         start=False, stop=True,
                                         skip_group_check=True)
                        ev(t * 2 + n2, [nc.vector, nc.scalar],
                           y_t[:, n2 * WIN:(n2 + 1) * WIN], p[:])
                    yq = nc.gpsimd if t % 2 == 0 else nc.sync
                    yq.dma_start(y[(t - 1) * 128:t * 128, :], y_t[:])

                    if dbg:
                        nc.vector.tensor_copy(zr8[:, t - 1:t], zrf[:])
                if dbg:
                    twf = pers.tile([1, 1024], F32, tag="dtw")
                    nc.vector.tensor_copy(twf[:], ci_r[:])
                    nc.sync.dma_start(d_ci[:], twf[:])
                    fwf = pers.tile([1, 1024], F32, tag="dfw")
                    nc.vector.tensor_copy(fwf[:], fwrow[:])
                    nc.sync.dma_start(d_fw[:], fwf[:])
                    nc.sync.dma_start(d_ts[:], ts_t[:])
                    nc.sync.dma_start(d_g8[:], g8[:])
                    nc.sync.dma_start(d_zr8[:], zr8[:])
                    nc.sync.dma_start(d_rs8[:], rs8[:])
                    vsf = pers.tile([1, H], F32, tag="dvs")
                    nc.vector.tensor_copy(vsf[:], vswo_b[:])
                    nc.sync.dma_start(d_vswo[:], vsf[:])
                    ktf = pers.tile([128, 128], F32, tag="dkt")
                    nc.vector.tensor_copy(ktf[:], kT8[:, 128:256])
                    nc.sync.dma_start(d_kt[:], ktf[:])
                    qf = pers.tile([128, 128], F32, tag="dq")
                    nc.vector.tensor_copy(qf[:], qT8[:, 0:128])
                    nc.sync.dma_start(d_q[:], qf[:])
                    vwf = pers.tile([128, H], F32, tag="dvw")
                    nc.vector.tensor_copy(vwf[:], vw[:, H:2 * H])
                    nc.sync.dma_start(d_vw[:], vwf[:])
                    x8f = pers.tile([128, 256], F32, tag="dx8")
                    nc.vector.tensor_copy(x8f[:], xp8[:, 0:256])
                    nc.sync.dma_start(d_x8[:], x8f[:])
                    wkf = pers.tile([128, 256], F32, tag="dwk")
                    nc.vector.tensor_copy(wkf[:], wk_t[:, 0:256])
                    nc.sync.dma_start(d_wk[:], wkf[:])
                    bpf = pers.tile([128, 384], F32, tag="dbp")
                    nc.vector.tensor_copy(bpf[:], bp8[:, 0:384])
                    nc.sync.dma_start(d_bp[:], bpf[:])

    nc.compile()
    return nc


def prep_inputs(x, task_id, pos_emb, Wq, bq, Wk, bk, Wv, bv, Wo, bo,
                task_table, Wt1, bt1, Wt2, bt2,
                Wc1, bc1, Wc2, bc2, Wc3, bc3,
                Wf1, bf1, Wf2, bf2):
    f = np.float32
    bf = ml_dtypes.bfloat16
    f8 = mybir.dt.np(F8)
    T = lambda a: np.ascontiguousarray(np.asarray(a, dtype=f).T)
    c8 = lambda a: np.clip(np.asarray(a, dtype=f), -240, 240).astype(f8)

    wvo = T(Wv) @ T(Wo)                     # (Wo Wv)^T
    smf = np.zeros((128, 26), f)
    smf[:, 0:6] = np.asarray(bq).reshape(HC, 128).T
    smf[:, 6:12] = np.asarray(bk).reshape(HC, 128).T
    smf[:, 12:15] = np.asarray(bc1).reshape(3, 128).T
    bc2p = np.zeros(256, f); bc2p[:192] = np.asarray(bc2)
    smf[:, 15:17] = bc2p.reshape(2, 128).T
    smf[:, 17:23] = np.asarray(bf1).reshape(HC, 128).T
    smf[:, 23] = float(np.asarray(bc3).reshape(-1)[0])
    smf[:, 24] = float(np.asarray(bf2).reshape(-1)[0])

    p_ = np.arange(128)[:, None]; c_ = np.arange(WIN)[None, :]
    pwm = (np.exp(-np.abs(128 + p_ - c_) / 2.0) / math.sqrt(H)).astype(f)

    wc2p = np.zeros((384, 256), f); wc2p[:, :192] = T(Wc2)
    wc3p = np.zeros((256,), f); wc3p[:192] = np.asarray(Wc3, dtype=f).reshape(-1)
    f8pk = np.zeros((128, 8), f)
    f8pk[:, 6:8] = (wc3p.reshape(2, 128).T) * WC3_SC

    wtep = np.zeros((64, H + 1), bf)
    wtep[:, 0:H] = T(Wt1).astype(bf)
    wf1p = np.zeros((3, H + 1), bf)
    wf1p[:, 0:H] = (T(Wf1) / S).astype(bf)
    wf1p[0, H] = bf(1.0)
    bvoo = (np.asarray(bv, f) @ np.asarray(Wo, f).T + np.asarray(bo, f)).reshape(H)

    shared = {
        "wq8": c8(T(Wq)), "wk8": c8(T(Wk)), "wvo8": c8(wvo * WVO_SC),
        "wvob": wvo.astype(bf),
        "wc18": c8(T(Wc1)), "wc28": c8(wc2p),
        "wt2b": T(Wt2).astype(bf), "wtep": wtep, "wf1p": wf1p,
        "f8p": c8(f8pk),
        "smf": smf,
    }

    x = np.asarray(x); pos_emb = np.asarray(pos_emb)
    task_table = np.asarray(task_table); task_id = np.asarray(task_id)
    in_maps = []
    for core in range(8):
        b, half = core // 2, core % 2
        g0 = 1024 * half - 128
        xp2 = np.zeros((NB * 128, 2 * H), bf)
        lo, hi = max(0, g0), min(S, g0 + NB * 128)
        xp2[lo - g0:hi - g0, 0:H] = x[b, lo:hi]
        xp2[lo - g0:hi - g0, H:2 * H] = pos_emb[0, lo:hi]
        pwf = pwm.copy(); pwl = pwm.copy()
        if half == 0:
            pwf[:, :128] = 0
        if half == 1:
            pwl[:, 256:] = 0
        C0 = 129 + 3 * WIN
        bfpk = np.zeros((128, C0 + 134 + 3 * H), bf)
        bfpk[:, 0:128] = np.eye(128, dtype=f).astype(bf)
        bfpk[:, 128] = bf(1.0)
        bfpk[:, 129:129 + WIN] = pwf.astype(bf)
        bfpk[:, 129 + WIN:129 + 2 * WIN] = pwm.astype(bf)
        bfpk[:, 129 + 2 * WIN:C0] = pwl.astype(bf)
        bfpk[:, C0:C0 + 6] = np.asarray(Wf2, f).reshape(HC, 128).T.astype(bf)
        bfpk[0, C0 + 6:C0 + 134] = bf(1.0)
        bfpk[0, C0 + 134:C0 + 134 + H] = bvoo.astype(bf)
        bfpk[0, C0 + 134 + H:C0 + 134 + 2 * H] = np.asarray(bt1, f).astype(bf)
        bfpk[0, C0 + 134 + 2 * H:C0 + 134 + 3 * H] = np.asarray(bt2, f).astype(bf)
        wte = wtep.copy()
        wte[:, H] = task_table[int(task_id[b])].astype(bf)
        m = dict(shared)
        m.update({"xp2": xp2, "bfp": bfpk, "wtep": wte})
        in_maps.append(m)
    return in_maps


class _Runner:
    """Compile the SPMD graph once and keep a reusable jitted callable."""

    def __init__(self, nc, n_cores=8):
        import jax
        from jax.sharding import Mesh, PartitionSpec
        from jax.experimental.shard_map import shard_map
        from concourse import bass2jax, mybir as _mb
        bass2jax.install_neuronx_cc_hook()
        self.nc = nc
        partition_name = (nc.partition_id_tensor.name
                          if nc.partition_id_tensor else None)
        in_names, out_names, out_avals, zero_outs = [], [], [], []
        for alloc in nc.m.functions[0].allocations:
            if not isinstance(alloc, _mb.MemoryLocationSet):
                continue
            name = alloc.memorylocations[0].name
            if alloc.kind == "ExternalInput":
                if name != partition_name:
                    in_names.append(name)
            elif alloc.kind == "ExternalOutput":
                shape = tuple(alloc.tensor_shape)
                dtype = _mb.dt.np(alloc.dtype)
                out_names.append(name)
                out_avals.append(jax.core.ShapedArray(shape, dtype))
                zero_outs.append(np.zeros(shape, dtype))
        self.in_names = list(in_names)
        self.out_names = out_names
        self.out_avals = out_avals
        self.zero_outs = zero_outs
        self.n_cores = n_cores
        n_params = len(self.in_names)
        all_in = list(self.in_names) + list(out_names)
        if partition_name is not None:
            all_in.append(partition_name)

        def _body(*args):
            operands = list(args)
            if partition_name is not None:
                operands.append(bass2jax.partition_id_tensor())
            outs = bass2jax._bass_exec_p.bind(
                *operands,
                out_avals=tuple(out_avals),
                in_names=tuple(all_in),
                out_names=tuple(out_names),
                lowering_input_output_aliases=(),
                sim_require_finite=True,
                sim_require_nnan=True,
                nc=nc,
            )
            return tuple(outs)

        devices = jax.devices()[:n_cores]
        mesh = Mesh(np.asarray(devices), ("core",))
        n_outs = len(out_names)
        in_specs = (PartitionSpec("core"),) * (n_params + n_outs)
        out_specs = (PartitionSpec("core"),) * n_outs
        self.fn = jax.jit(
            shard_map(_body, mesh=mesh, in_specs=in_specs,
                      out_specs=out_specs, check_rep=False),
            keep_unused=True)

    def concat_inputs(self, in_maps):
        return [np.concatenate([np.asarray(in_maps[c][k])
                                for c in range(self.n_cores)], axis=0)
                for k in self.in_names]

    def zeros(self):
        return [np.zeros((self.n_cores * z.shape[0],) + z.shape[1:], z.dtype)
                for z in self.zero_outs]

    def __call__(self, concat_in, zeros=None):
        import jax
        if zeros is None:
            zeros = self.zeros()
        outs = jax.block_until_ready(self.fn(*concat_in, *zeros))
        return outs


def get_runner(dbg=False):
    key = "runner_dbg" if dbg else "runner"
    if key not in _cache:
        _cache[key] = _Runner(build_kernel(dbg=dbg))
    return _cache[key]


def kernel(**inputs):
    rn = get_runner()
    in_maps = prep_inputs(**inputs)
    outs = rn(rn.concat_inputs(in_maps))
    yc = np.asarray(outs[rn.out_names.index("y")]).reshape(8, 1024, H)
    out = np.zeros((B, S, H), np.float32)
    for core in range(8):
        b, half = core // 2, core % 2
        out[b, 1024 * half:1024 * (half + 1)] = yc[core]
    return out
